# revision 2
# baseline (speedup 1.0000x reference)
"""Trainium2 Bass kernel for nn_Head (single-head causal self-attention).

Module:  q = x@Wq.T, k = x@Wk.T, v = x@Wv.T
         wei = softmax(causal_mask(q@k.T * E**-0.5))
         out = wei @ v
Shapes:  x [2048, 128, 192], Wq/Wk/Wv [192, 192] -> out [2048, 128, 192]

Strategy (pure data parallel over the batch dim, 8 cores x 256 batches):
  - Weight fold: wei = x @ A @ x.T with A = (Wq.T @ Wk) * SCALE, so only one
    projection ("g = x @ A") is needed for the attention logits.
  - Host prepares x transposed per-core as xt[e, b*T + t] in bf16 (layout +
    dtype prep only; all model FLOPs run on device).
  - Per batch on device:  gT = A.T @ xT (A-stationary, 4-batch column blocks),
    wei = gT.T @ xT, P = exp(wei) (ACT), Pm = P*mask with row-sum (DVE TTR),
    Pm *= 1/s, PT = transpose(Pm) (PE), v = xT.T @ Wv.T, o = PT.T @ v.
"""

import os
import sys

sys.path.insert(0, "/opt/trn_rl_repo")

import numpy as np
import ml_dtypes
from contextlib import ExitStack

import json

import concourse.bass as bass
import concourse.bass2jax as bass2jax
import concourse.mybir as mybir
import concourse.tile as tile
from concourse.bass_utils import (
    compile_bir_kernel as _orig_compile_bir_kernel,
    run_bass_kernel_spmd,
)

BF16 = mybir.dt.bfloat16
F32 = mybir.dt.float32
NPBF16 = ml_dtypes.bfloat16

B, T, E, H = 2048, 128, 192, 192
NCORES = 8
NB = B // NCORES            # batches per core
SCALE = float(E) ** -0.5
G = 8                       # batches per DMA group
QUAD = 4                    # batches sharing one PSUM bank for wei/PT
NGROUPS = NB // G


def _patch_tile_tail_drain():
    """Walrus rejects the TileContext tail Drain when it carries more than a
    couple of sem waits ("Too many sync wait commands").  Redistribute the
    waits onto single-wait SP nops emitted between the drain and barrier."""
    if getattr(tile.TileContext, "_tail_drain_patched", False):
        return

    def _drain_and_barrier(self, tick_clock, wait_clock):
        from concourse.tile import ScopedClock

        drain_inst = self.nc.sync.drain()
        wait_clock.add_sem_waits(
            drain_inst.ins, ScopedClock({None: tick_clock.global_clock})
        )
        waits = list(drain_inst.ins.sync_info.on_wait or [])
        if len(waits) > 1:
            drain_inst.ins.sync_info = mybir.SyncInfo(
                on_wait=[waits[0]], on_update=[]
            )
            for w in waits[1:]:
                nop = self.nc.sync.nop()
                nop.ins.sync_info = mybir.SyncInfo(on_wait=[w], on_update=[])
        self.nc.all_engine_barrier()
        assert self.sems is not None
        popped = self.nc._tile_sem_poison_stack.pop()
        assert popped is self._sem_poison
        self.nc.clear_and_free_semaphores(list(self.sems.allocated().values()))
        self.nc.all_engine_barrier()

    tile.TileContext._drain_and_barrier = _drain_and_barrier
    tile.TileContext._tail_drain_patched = True


def _split_multi_waits(bir_json: bytes) -> bytes:
    """This container's walrus supports only ONE sync-wait slot per
    instruction ("Too many sync wait commands").  Hoist extra waits onto
    single-wait NoOps inserted just before the instruction (same engine, so
    per-engine program order and blocking semantics are preserved)."""
    d = json.loads(bir_json)
    n = 0
    for f in d.get("functions", []):
        for bb in f.get("blocks", []):
            insts = bb.get("instructions", [])
            out = []
            changed = False
            for inst in insts:
                si = inst.get("sync_info")
                waits = (si.get("on_wait") or []) if si else []
                if len(waits) > 1:
                    changed = True
                    for w in waits[:-1]:
                        n += 1
                        out.append({
                            "debug": inst.get("debug"),
                            "engine": inst["engine"],
                            "ins": [],
                            "name": f"WSPLIT-{n}",
                            "opcode": "NoOp",
                            "outs": [],
                            "sync_info": {"on_update": [], "on_wait": [w]},
                        })
                    si["on_wait"] = [waits[-1]]
                out.append(inst)
            if changed:
                bb["instructions"] = out
    if n == 0:
        return bir_json
    return json.dumps(d).encode()


def _patched_compile_bir_kernel(bir_json, tmpdir, neff_name="file.neff"):
    if isinstance(bir_json, str):
        bir_json = bir_json.encode()
    return _orig_compile_bir_kernel(_split_multi_waits(bir_json), tmpdir, neff_name)


bass2jax.compile_bir_kernel = _patched_compile_bir_kernel


def build_nc(nb=NB):
    _patch_tile_tail_drain()
    nc = bass.Bass(trn_type="TRN2")

    xt = nc.dram_tensor("xt", [E, nb * T], BF16, kind="ExternalInput")
    a = nc.dram_tensor("a", [E, E], BF16, kind="ExternalInput")
    wvt = nc.dram_tensor("wvt", [E, H], BF16, kind="ExternalInput")
    o = nc.dram_tensor("o", [nb, T, H], F32, kind="ExternalOutput")

    ngroups = nb // G
    mult = mybir.AluOpType.mult
    add = mybir.AluOpType.add

    with tile.TileContext(nc) as tc, ExitStack() as ctx:
        singles = ctx.enter_context(tc.tile_pool(name="singles", bufs=1))
        px = ctx.enter_context(tc.tile_pool(name="px", bufs=3))
        pgsb = ctx.enter_context(tc.tile_pool(name="pgsb", bufs=2))
        pp = ctx.enter_context(tc.tile_pool(name="pp", bufs=3))
        psr = ctx.enter_context(tc.tile_pool(name="psr", bufs=4))
        pptsb = ctx.enter_context(tc.tile_pool(name="pptsb", bufs=3))
        pvsb = ctx.enter_context(tc.tile_pool(name="pvsb", bufs=6))
        posb = ctx.enter_context(tc.tile_pool(name="posb", bufs=3))

        pglo = ctx.enter_context(tc.tile_pool(name="pglo", bufs=1, space="PSUM"))
        pghi = ctx.enter_context(tc.tile_pool(name="pghi", bufs=1, space="PSUM"))
        pw = ctx.enter_context(tc.tile_pool(name="pw", bufs=1, space="PSUM"))
        ppt = ctx.enter_context(tc.tile_pool(name="ppt", bufs=1, space="PSUM"))
        pv = ctx.enter_context(tc.tile_pool(name="pv", bufs=2, space="PSUM"))
        po = ctx.enter_context(tc.tile_pool(name="po", bufs=2, space="PSUM"))

        # Constants: A (lhsT for gT), WvT (rhs for v), identity, causal mask.
        a_lo = singles.tile([128, E], BF16, tag="a_lo")
        a_hi = singles.tile([64, E], BF16, tag="a_hi")
        nc.sync.dma_start(out=a_lo, in_=a[0:128, :])
        nc.sync.dma_start(out=a_hi, in_=a[128:192, :])
        wvt_lo = singles.tile([128, H], BF16, tag="wvt_lo")
        wvt_hi = singles.tile([64, H], BF16, tag="wvt_hi")
        nc.sync.dma_start(out=wvt_lo, in_=wvt[0:128, :])
        nc.sync.dma_start(out=wvt_hi, in_=wvt[128:192, :])

        ident = singles.tile([128, 128], BF16, tag="ident")
        nc.gpsimd.memset(ident, 0.0)
        nc.gpsimd.affine_select(
            out=ident, in_=ident,
            compare_op=mybir.AluOpType.not_equal,
            fill=1.0, base=0, pattern=[[-1, 128]], channel_multiplier=1,
        )
        # mask4[q, g, k] = 1.0 if k <= q else 0.0  (causal mask, tiled QUAD x)
        mask4 = singles.tile([128, QUAD, 128], BF16, tag="mask4")
        nc.gpsimd.memset(mask4, 1.0)
        nc.gpsimd.affine_select(
            out=mask4, in_=mask4,
            compare_op=mybir.AluOpType.is_ge,
            fill=0.0, base=0, pattern=[[0, QUAD], [-1, 128]], channel_multiplier=1,
        )

        # Software pipeline over quads: at iteration Q emit
        #   gT(Q), v(Q)  ->  PT(Q-1)  ->  wei(Q)  ->  o(Q-2)
        # so PE never waits on the vector-side chain exp -> mask -> copy.
        nq = nb // QUAD
        x_tiles = {}     # group -> (xlo, xhi)
        gsb_t = {}       # Q -> (gsb_lo, gsb_hi)
        pm_t = {}        # Q -> pm
        ptsb_t = {}      # Q -> pt_sb
        vsb_t = {}       # Q -> [v_sb pair0, v_sb pair1]
        osb_t = {}       # group -> o_sb

        for Q in range(nq + 2):
            if Q < nq:
                g = Q * QUAD // G
                if (Q * QUAD) % G == 0:
                    gcol = g * G * T
                    xlo = px.tile([128, G * T], BF16, tag="xlo")
                    xhi = px.tile([64, G * T], BF16, tag="xhi")
                    nc.sync.dma_start(out=xlo, in_=xt[0:128, gcol : gcol + G * T])
                    nc.sync.dma_start(out=xhi, in_=xt[128:192, gcol : gcol + G * T])
                    x_tiles[g] = (xlo, xhi)
                xlo, xhi = x_tiles[g]
                qs = (Q * QUAD * T) % (G * T)
                qcols = slice(qs, qs + QUAD * T)

                # gT = A.T @ xT for 4 batches (N=512)
                glo = pglo.tile([128, QUAD * T], F32, tag="glo")
                ghi = pghi.tile([64, QUAD * T], F32, tag="ghi")
                nc.tensor.matmul(glo, a_lo[:, 0:128], xlo[:, qcols],
                                 start=True, stop=False)
                nc.tensor.matmul(glo, a_hi[:, 0:128], xhi[:, qcols],
                                 start=False, stop=True)
                nc.tensor.matmul(ghi, a_lo[:, 128:192], xlo[:, qcols],
                                 start=True, stop=False)
                nc.tensor.matmul(ghi, a_hi[:, 128:192], xhi[:, qcols],
                                 start=False, stop=True)
                gsb_lo = pgsb.tile([128, QUAD * T], BF16, tag="gsb_lo")
                gsb_hi = pgsb.tile([64, QUAD * T], BF16, tag="gsb_hi")
                nc.scalar.copy(out=gsb_lo, in_=glo)
                nc.vector.tensor_copy(out=gsb_hi, in_=ghi)
                gsb_t[Q] = (gsb_lo, gsb_hi)

                # v = xT.T @ WvT, two batches per PSUM bank; v_ext = [v | 1]
                vsb_t[Q] = []
                for pr in range(QUAD // 2):
                    v_ps = pv.tile([128, 2, H], F32, tag="v_ps")
                    for jj in range(2):
                        bs = qs + (pr * 2 + jj) * T
                        nc.tensor.matmul(v_ps[:, jj, :], xlo[:, bs : bs + T],
                                         wvt_lo, start=True, stop=False)
                        nc.tensor.matmul(v_ps[:, jj, :], xhi[:, bs : bs + T],
                                         wvt_hi, start=False, stop=True)
                    v_sb = pvsb.tile([128, 2, H + 8], BF16, tag="v_sb")
                    nc.scalar.copy(out=v_sb[:, :, 0:H], in_=v_ps)
                    nc.gpsimd.memset(v_sb[:, :, H : H + 1], 1.0)
                    vsb_t[Q].append(v_sb)

            # PT(Q-1) = transpose(Pm(Q-1))
            if 1 <= Q <= nq:
                pm = pm_t.pop(Q - 1)
                pt_ps = ppt.tile([128, QUAD, T], BF16, tag="pt_ps")
                for j in range(QUAD):
                    nc.tensor.transpose(pt_ps[:, j, :], pm[:, j, :], ident)
                pt_sb = pptsb.tile([128, QUAD, T], BF16, tag="pt_sb")
                nc.vector.tensor_copy(out=pt_sb, in_=pt_ps)
                ptsb_t[Q - 1] = pt_sb

            if Q < nq:
                # wei[j] = gT_j.T @ xT_j  (one PSUM bank per quad)
                gsb_lo, gsb_hi = gsb_t.pop(Q)
                wei = pw.tile([128, QUAD, T], F32, tag="wei")
                for j in range(QUAD):
                    bs = qs + j * T
                    jc = slice(j * T, (j + 1) * T)
                    nc.tensor.matmul(wei[:, j, :], gsb_lo[:, jc],
                                     xlo[:, bs : bs + T], start=True, stop=False)
                    nc.tensor.matmul(wei[:, j, :], gsb_hi[:, jc],
                                     xhi[:, bs : bs + T], start=False, stop=True)

                # P = exp(wei) (ACT) ; Pm = P * causal_mask (DVE)
                p_sb = pp.tile([128, QUAD, T], BF16, tag="p_sb")
                nc.scalar.activation(out=p_sb, in_=wei,
                                     func=mybir.ActivationFunctionType.Exp)
                pm = pp.tile([128, QUAD, T], BF16, tag="pm")
                nc.vector.tensor_mul(pm, p_sb, mask4)
                pm_t[Q] = pm

            # o(Q-2) = PT.T @ v_ext ; col H = softmax denominator
            if Q >= 2:
                oq = Q - 2
                gb = oq * QUAD // G
                ob0 = (oq * QUAD) % G
                if ob0 == 0:
                    osb_t[gb] = posb.tile([128, G, H], F32, tag="o_sb",
                                          name="o_sb")
                o_sb = osb_t[gb]
                pt_sb = ptsb_t.pop(oq)
                for pr in range(QUAD // 2):
                    v_sb = vsb_t[oq][pr]
                    o_ps = po.tile([128, 2, H + 8], F32, tag="o_ps")
                    for jj in range(2):
                        j = pr * 2 + jj
                        nc.tensor.matmul(o_ps[:, jj, 0 : H + 1], pt_sb[:, j, :],
                                         v_sb[:, jj, 0 : H + 1],
                                         start=True, stop=True)
                    r = psr.tile([128, 2], F32, tag="r")
                    nc.vector.reciprocal(out=r, in_=o_ps[:, :, H])
                    ob = ob0 + pr * 2
                    nc.vector.tensor_scalar_mul(
                        out=o_sb[:, ob, :], in0=o_ps[:, 0, 0:H],
                        scalar1=r[:, 0:1],
                    )
                    nc.scalar.mul(
                        out=o_sb[:, ob + 1, :], in_=o_ps[:, 1, 0:H],
                        mul=r[:, 1:2],
                    )
                del vsb_t[oq]
                if ob0 + QUAD == G:
                    nc.sync.dma_start(
                        out=o[gb * G : (gb + 1) * G, :, :].rearrange(
                            "b t h -> t b h"
                        ),
                        in_=o_sb,
                    )
                    del osb_t[gb]
    return nc


_cached = {}


def _get_nc(nb):
    if nb not in _cached:
        _cached[nb] = build_nc(nb)
    return _cached[nb]


def prep_inputs(x, Wq, Wk, Wv, nb=NB, ncores=NCORES):
    """Host-side sharding + layout/dtype prep + weight folding."""
    x = np.asarray(x, dtype=np.float32)
    A = (np.asarray(Wq, np.float32).T @ np.asarray(Wk, np.float32)) * SCALE
    a_bf = np.ascontiguousarray(A).astype(NPBF16)
    wvt_bf = np.ascontiguousarray(np.asarray(Wv, np.float32).T).astype(NPBF16)
    in_maps = []
    for c in range(ncores):
        shard = x[c * nb : (c + 1) * nb]                      # [nb, T, E]
        xt = np.ascontiguousarray(shard.transpose(2, 0, 1)).reshape(E, nb * T)
        in_maps.append({"xt": xt.astype(NPBF16), "a": a_bf, "wvt": wvt_bf})
    return in_maps


def kernel(x, Wq, Wk, Wv, _trace=False, _tmpdir=None):
    nc = _get_nc(NB)
    in_maps = prep_inputs(x, Wq, Wk, Wv)
    res = run_bass_kernel_spmd(
        nc, in_maps, core_ids=list(range(NCORES)), trace=_trace, tmpdir=_tmpdir
    )
    out = np.concatenate([res.results[c]["o"] for c in range(NCORES)], axis=0)
    if _trace:
        kernel.last_result = res
    return out



# revision 6
# speedup vs baseline: 2.7931x; 2.7931x over previous
"""Trainium2 Bass kernel for nn_Head (single-head causal self-attention).

Module:  q = x@Wq.T, k = x@Wk.T, v = x@Wv.T
         wei = softmax(causal_mask(q@k.T * E**-0.5))
         out = wei @ v
Shapes:  x [2048, 128, 192], Wq/Wk/Wv [192, 192] -> out [2048, 128, 192]

Strategy (pure data parallel over the batch dim, 8 cores x 256 batches):
  - Weight fold: wei = x @ A @ x.T with A = (Wq.T @ Wk) * SCALE, so only one
    projection ("g = x @ A") is needed for the attention logits.
  - Host prepares x transposed per-core as xt[e, b*T + t] in bf16 (layout +
    dtype prep only; all model FLOPs run on device).
  - Per batch on device:  gT = A.T @ xT (A-stationary, 4-batch column blocks),
    wei = gT.T @ xT, P = exp(wei) (ACT), Pm = P*mask with row-sum (DVE TTR),
    Pm *= 1/s, PT = transpose(Pm) (PE), v = xT.T @ Wv.T, o = PT.T @ v.
"""

import os
import sys

sys.path.insert(0, "/opt/trn_rl_repo")

import numpy as np
import ml_dtypes
from contextlib import ExitStack

import json

import concourse.bass as bass
import concourse.bass2jax as bass2jax
import concourse.mybir as mybir
import concourse.tile as tile
from concourse.bass_utils import (
    compile_bir_kernel as _orig_compile_bir_kernel,
    run_bass_kernel_spmd,
)

BF16 = mybir.dt.bfloat16
F32 = mybir.dt.float32
NPBF16 = ml_dtypes.bfloat16

B, T, E, H = 2048, 128, 192, 192
NCORES = 8
NB = B // NCORES            # batches per core
SCALE = float(E) ** -0.5
G = 8                       # batches per DMA group
QUAD = 4                    # batches sharing one PSUM bank for wei/PT
NGROUPS = NB // G


def _patch_tile_tail_drain():
    """Walrus rejects the TileContext tail Drain when it carries more than a
    couple of sem waits ("Too many sync wait commands").  Redistribute the
    waits onto single-wait SP nops emitted between the drain and barrier."""
    if getattr(tile.TileContext, "_tail_drain_patched", False):
        return

    def _drain_and_barrier(self, tick_clock, wait_clock):
        from concourse.tile import ScopedClock

        drain_inst = self.nc.sync.drain()
        wait_clock.add_sem_waits(
            drain_inst.ins, ScopedClock({None: tick_clock.global_clock})
        )
        waits = list(drain_inst.ins.sync_info.on_wait or [])
        if len(waits) > 1:
            drain_inst.ins.sync_info = mybir.SyncInfo(
                on_wait=[waits[0]], on_update=[]
            )
            for w in waits[1:]:
                nop = self.nc.sync.nop()
                nop.ins.sync_info = mybir.SyncInfo(on_wait=[w], on_update=[])
        self.nc.all_engine_barrier()
        assert self.sems is not None
        popped = self.nc._tile_sem_poison_stack.pop()
        assert popped is self._sem_poison
        self.nc.clear_and_free_semaphores(list(self.sems.allocated().values()))
        self.nc.all_engine_barrier()

    tile.TileContext._drain_and_barrier = _drain_and_barrier
    tile.TileContext._tail_drain_patched = True


def _split_multi_waits(bir_json: bytes) -> bytes:
    """This container's walrus supports only ONE sync-wait slot per
    instruction ("Too many sync wait commands").  Hoist extra waits onto
    single-wait NoOps inserted just before the instruction (same engine, so
    per-engine program order and blocking semantics are preserved)."""
    d = json.loads(bir_json)
    n = 0
    for f in d.get("functions", []):
        for bb in f.get("blocks", []):
            insts = bb.get("instructions", [])
            out = []
            changed = False
            for inst in insts:
                si = inst.get("sync_info")
                waits = (si.get("on_wait") or []) if si else []
                if len(waits) > 1:
                    changed = True
                    for w in waits[:-1]:
                        n += 1
                        out.append({
                            "debug": inst.get("debug"),
                            "engine": inst["engine"],
                            "ins": [],
                            "name": f"WSPLIT-{n}",
                            "opcode": "NoOp",
                            "outs": [],
                            "sync_info": {"on_update": [], "on_wait": [w]},
                        })
                    si["on_wait"] = [waits[-1]]
                out.append(inst)
            if changed:
                bb["instructions"] = out
    if n == 0:
        return bir_json
    return json.dumps(d).encode()


def _patched_compile_bir_kernel(bir_json, tmpdir, neff_name="file.neff"):
    if isinstance(bir_json, str):
        bir_json = bir_json.encode()
    return _orig_compile_bir_kernel(_split_multi_waits(bir_json), tmpdir, neff_name)


bass2jax.compile_bir_kernel = _patched_compile_bir_kernel

if os.environ.get("KLDW", "0") == "1":
    import concourse.bass_utils as _bu

    _orig_run_command = _bu.run_command

    def _ldw_run_command(argv, **kwargs):
        argv = [
            a.replace("--enable-ldw-opt=false", "--enable-ldw-opt=true")
            for a in argv
        ]
        return _orig_run_command(argv, **kwargs)

    _bu.run_command = _ldw_run_command


def build_nc(nb=NB):
    _patch_tile_tail_drain()
    nc = bass.Bass(trn_type="TRN2")

    xt = nc.dram_tensor("xt", [E, nb * T], BF16, kind="ExternalInput")
    a = nc.dram_tensor("a", [E, E], BF16, kind="ExternalInput")
    wvt = nc.dram_tensor("wvt", [E, H], BF16, kind="ExternalInput")
    o = nc.dram_tensor("o", [nb, T, H], F32, kind="ExternalOutput")

    ngroups = nb // G
    mult = mybir.AluOpType.mult
    add = mybir.AluOpType.add

    with tile.TileContext(nc) as tc, ExitStack() as ctx:
        singles = ctx.enter_context(tc.tile_pool(name="singles", bufs=1))
        px = ctx.enter_context(tc.tile_pool(name="px", bufs=3))
        pgsb = ctx.enter_context(tc.tile_pool(name="pgsb", bufs=2))
        pp = ctx.enter_context(tc.tile_pool(name="pp", bufs=3))
        psr = ctx.enter_context(tc.tile_pool(name="psr", bufs=4))
        pptsb = ctx.enter_context(tc.tile_pool(name="pptsb", bufs=3))
        pvsb = ctx.enter_context(tc.tile_pool(name="pvsb", bufs=6))
        posb = ctx.enter_context(tc.tile_pool(name="posb", bufs=3))

        pglo = ctx.enter_context(tc.tile_pool(name="pglo", bufs=1, space="PSUM"))
        pghi = ctx.enter_context(tc.tile_pool(name="pghi", bufs=1, space="PSUM"))
        pw = ctx.enter_context(tc.tile_pool(name="pw", bufs=1, space="PSUM"))
        ppt = ctx.enter_context(tc.tile_pool(name="ppt", bufs=1, space="PSUM"))
        pv = ctx.enter_context(tc.tile_pool(name="pv", bufs=2, space="PSUM"))
        po = ctx.enter_context(tc.tile_pool(name="po", bufs=2, space="PSUM"))

        # Constants: A (lhsT for gT), WvT (rhs for v), identity, causal mask.
        a_lo = singles.tile([128, E], BF16, tag="a_lo")
        a_hi = singles.tile([64, E], BF16, tag="a_hi")
        nc.sync.dma_start(out=a_lo, in_=a[0:128, :])
        nc.sync.dma_start(out=a_hi, in_=a[128:192, :])
        wvt_lo = singles.tile([128, H], BF16, tag="wvt_lo")
        wvt_hi = singles.tile([64, H], BF16, tag="wvt_hi")
        nc.sync.dma_start(out=wvt_lo, in_=wvt[0:128, :])
        nc.sync.dma_start(out=wvt_hi, in_=wvt[128:192, :])

        ident = singles.tile([128, 128], BF16, tag="ident")
        nc.gpsimd.memset(ident, 0.0)
        nc.gpsimd.affine_select(
            out=ident, in_=ident,
            compare_op=mybir.AluOpType.not_equal,
            fill=1.0, base=0, pattern=[[-1, 128]], channel_multiplier=1,
        )
        # mask4[q, g, k] = 1.0 if k <= q else 0.0  (causal mask, tiled QUAD x)
        mask4 = singles.tile([128, QUAD, 128], BF16, tag="mask4")
        nc.gpsimd.memset(mask4, 1.0)
        nc.gpsimd.affine_select(
            out=mask4, in_=mask4,
            compare_op=mybir.AluOpType.is_ge,
            fill=0.0, base=0, pattern=[[0, QUAD], [-1, 128]], channel_multiplier=1,
        )

        nwarm = int(os.environ.get("KWARM", "0"))

        # Software pipeline over quads: at iteration Q emit
        #   gT(Q), v(Q)  ->  PT(Q-1)  ->  wei(Q)  ->  o(Q-2)
        # so PE never waits on the vector-side chain exp -> mask -> copy.
        nq = nb // QUAD
        x_tiles = {}     # group -> (xlo, xhi)
        gsb_t = {}       # Q -> (gsb_lo, gsb_hi)
        pm_t = {}        # Q -> pm
        ptsb_t = {}      # Q -> pt_sb
        vsb_t = {}       # Q -> [v_sb pair0, v_sb pair1]
        osb_t = {}       # group -> o_sb

        for Q in range(nq + 2):
            if Q < nq:
                g = Q * QUAD // G
                if (Q * QUAD) % G == 0:
                    gcol = g * G * T
                    xlo = px.tile([128, G * T], BF16, tag="xlo")
                    xhi = px.tile([64, G * T], BF16, tag="xhi")
                    nc.sync.dma_start(out=xlo, in_=xt[0:128, gcol : gcol + G * T])
                    nc.sync.dma_start(out=xhi, in_=xt[128:192, gcol : gcol + G * T])
                    x_tiles[g] = (xlo, xhi)
                xlo, xhi = x_tiles[g]
                qs = (Q * QUAD * T) % (G * T)
                qcols = slice(qs, qs + QUAD * T)

                # gT = A.T @ xT for 4 batches (N=512)
                glo = pglo.tile([128, QUAD * T], F32, tag="glo")
                ghi = pghi.tile([64, QUAD * T], F32, tag="ghi")
                if Q == 0 and nwarm:
                    # HAM warmup: dense burst of N=512 matmuls (overwritten
                    # by the real gT below) to push the PE clock to 2.4 GHz.
                    for _ in range(nwarm):
                        nc.tensor.matmul(glo, a_lo[:, 0:128], xlo[:, qcols],
                                         start=True, stop=True)
                nc.tensor.matmul(glo, a_lo[:, 0:128], xlo[:, qcols],
                                 start=True, stop=False)
                nc.tensor.matmul(glo, a_hi[:, 0:128], xhi[:, qcols],
                                 start=False, stop=True)
                nc.tensor.matmul(ghi, a_lo[:, 128:192], xlo[:, qcols],
                                 start=True, stop=False)
                nc.tensor.matmul(ghi, a_hi[:, 128:192], xhi[:, qcols],
                                 start=False, stop=True)
                gsb_lo = pgsb.tile([128, QUAD * T], BF16, tag="gsb_lo")
                gsb_hi = pgsb.tile([64, QUAD * T], BF16, tag="gsb_hi")
                nc.scalar.copy(out=gsb_lo, in_=glo)
                nc.vector.tensor_copy(out=gsb_hi, in_=ghi)
                gsb_t[Q] = (gsb_lo, gsb_hi)

                # v = xT.T @ WvT, two batches per PSUM bank; v_ext = [v | 1]
                vsb_t[Q] = []
                for pr in range(QUAD // 2):
                    v_ps = pv.tile([128, 2, H], F32, tag="v_ps")
                    for jj in range(2):
                        bs = qs + (pr * 2 + jj) * T
                        nc.tensor.matmul(v_ps[:, jj, :], xlo[:, bs : bs + T],
                                         wvt_lo, start=True, stop=False)
                        nc.tensor.matmul(v_ps[:, jj, :], xhi[:, bs : bs + T],
                                         wvt_hi, start=False, stop=True)
                    v_sb = pvsb.tile([128, 2, H + 8], BF16, tag="v_sb")
                    nc.scalar.copy(out=v_sb[:, :, 0:H], in_=v_ps)
                    nc.gpsimd.memset(v_sb[:, :, H : H + 1], 1.0)
                    vsb_t[Q].append(v_sb)

            # PT(Q-1) = transpose(Pm(Q-1))
            if 1 <= Q <= nq:
                pm = pm_t.pop(Q - 1)
                pt_ps = ppt.tile([128, QUAD, T], BF16, tag="pt_ps")
                for j in range(QUAD):
                    nc.tensor.transpose(pt_ps[:, j, :], pm[:, j, :], ident)
                pt_sb = pptsb.tile([128, QUAD, T], BF16, tag="pt_sb")
                nc.vector.tensor_copy(out=pt_sb, in_=pt_ps)
                ptsb_t[Q - 1] = pt_sb

            if Q < nq:
                # wei[j] = gT_j.T @ xT_j  (one PSUM bank per quad)
                gsb_lo, gsb_hi = gsb_t.pop(Q)
                wei = pw.tile([128, QUAD, T], F32, tag="wei")
                for j in range(QUAD):
                    bs = qs + j * T
                    jc = slice(j * T, (j + 1) * T)
                    nc.tensor.matmul(wei[:, j, :], gsb_lo[:, jc],
                                     xlo[:, bs : bs + T], start=True, stop=False)
                    nc.tensor.matmul(wei[:, j, :], gsb_hi[:, jc],
                                     xhi[:, bs : bs + T], start=False, stop=True)

                # P = exp(wei) (ACT) ; Pm = P * causal_mask (DVE)
                p_sb = pp.tile([128, QUAD, T], BF16, tag="p_sb")
                nc.scalar.activation(out=p_sb, in_=wei,
                                     func=mybir.ActivationFunctionType.Exp)
                pm = pp.tile([128, QUAD, T], BF16, tag="pm")
                nc.vector.tensor_mul(pm, p_sb, mask4)
                pm_t[Q] = pm

            # o(Q-2) = PT.T @ v_ext ; col H = softmax denominator
            if Q >= 2:
                oq = Q - 2
                gb = oq * QUAD // G
                ob0 = (oq * QUAD) % G
                if ob0 == 0:
                    osb_t[gb] = posb.tile([128, G, H], F32, tag="o_sb",
                                          name="o_sb")
                o_sb = osb_t[gb]
                pt_sb = ptsb_t.pop(oq)
                for pr in range(QUAD // 2):
                    v_sb = vsb_t[oq][pr]
                    o_ps = po.tile([128, 2, H + 8], F32, tag="o_ps")
                    for jj in range(2):
                        j = pr * 2 + jj
                        nc.tensor.matmul(o_ps[:, jj, 0 : H + 1], pt_sb[:, j, :],
                                         v_sb[:, jj, 0 : H + 1],
                                         start=True, stop=True)
                    r = psr.tile([128, 2], F32, tag="r")
                    nc.vector.reciprocal(out=r, in_=o_ps[:, :, H])
                    ob = ob0 + pr * 2
                    nc.vector.tensor_scalar_mul(
                        out=o_sb[:, ob, :], in0=o_ps[:, 0, 0:H],
                        scalar1=r[:, 0:1],
                    )
                    nc.scalar.mul(
                        out=o_sb[:, ob + 1, :], in_=o_ps[:, 1, 0:H],
                        mul=r[:, 1:2],
                    )
                del vsb_t[oq]
                if ob0 + QUAD == G:
                    nc.sync.dma_start(
                        out=o[gb * G : (gb + 1) * G, :, :].rearrange(
                            "b t h -> t b h"
                        ),
                        in_=o_sb,
                    )
                    del osb_t[gb]
    return nc


_cached = {}


def _get_nc(nb):
    if nb not in _cached:
        _cached[nb] = build_nc(nb)
    return _cached[nb]


def prep_inputs(x, Wq, Wk, Wv, nb=NB, ncores=NCORES):
    """Host-side sharding + layout/dtype prep + weight folding."""
    x = np.asarray(x, dtype=np.float32)
    A = (np.asarray(Wq, np.float32).T @ np.asarray(Wk, np.float32)) * SCALE
    a_bf = np.ascontiguousarray(A).astype(NPBF16)
    wvt_bf = np.ascontiguousarray(np.asarray(Wv, np.float32).T).astype(NPBF16)
    in_maps = []
    for c in range(ncores):
        shard = x[c * nb : (c + 1) * nb]                      # [nb, T, E]
        xt = np.ascontiguousarray(shard.transpose(2, 0, 1)).reshape(E, nb * T)
        in_maps.append({"xt": xt.astype(NPBF16), "a": a_bf, "wvt": wvt_bf})
    return in_maps


def kernel(x, Wq, Wk, Wv, _trace=False, _tmpdir=None):
    nc = _get_nc(NB)
    in_maps = prep_inputs(x, Wq, Wk, Wv)
    res = run_bass_kernel_spmd(
        nc, in_maps, core_ids=list(range(NCORES)), trace=_trace, tmpdir=_tmpdir
    )
    out = np.concatenate([res.results[c]["o"] for c in range(NCORES)], axis=0)
    if _trace:
        kernel.last_result = res
    return out



# revision 7
# speedup vs baseline: 2.8855x; 1.0331x over previous
"""Trainium2 Bass kernel v2 for nn_Head (single-head causal self-attention).

Module:  q = x@Wq.T, k = x@Wk.T, v = x@Wv.T
         wei = softmax(causal_mask(q@k.T * E**-0.5))
         out = wei @ v
Shapes:  x [2048, 128, 192], Wq/Wk/Wv [192, 192] -> out [2048, 128, 192]

v2 strategy (vs v1 baseline):
  - weiT computed directly (weiT[k,q] = xT.T @ gT) so the PE transpose of
    the attention matrix disappears; pm = exp(weiT) feeds o-matmul lhsT.
  - causal mask applied in-place by GPSIMD affine_select (frees DVE).
  - EVERY matmul is K=128 / M=128: the E=192 contraction is zero-padded to
    256 (K=64 operands serialize LDWEIGHTS pull-ahead and keep the PE HAM
    clock-gate cold; full-K chains run back-to-back at N/2.4 with hidden
    weight loads).
  - o normalized by one broadcast tensor_mul per quad; shipped bf16 in
    [T, nb, H] layout (host transposes + casts).
"""

import os
import sys

sys.path.insert(0, "/opt/trn_rl_repo")

import numpy as np
import ml_dtypes
from contextlib import ExitStack

import json

import concourse.bass as bass
import concourse.bass2jax as bass2jax
import concourse.mybir as mybir
import concourse.tile as tile
from concourse.bass_utils import (
    compile_bir_kernel as _orig_compile_bir_kernel,
    run_bass_kernel_spmd,
)

BF16 = mybir.dt.bfloat16
F32 = mybir.dt.float32
NPBF16 = ml_dtypes.bfloat16

B, T, E, H = 2048, 128, 192, 192
NCORES = 8
NB = B // NCORES            # batches per core
SCALE = float(E) ** -0.5
G = 8                       # batches per DMA group
QUAD = 4                    # batches per PSUM bank of weiT
HP = H + 8                  # v/o PSUM row pitch


def _patch_tile_tail_drain():
    """Walrus rejects the TileContext tail Drain when it carries more than a
    couple of sem waits ("Too many sync wait commands").  Redistribute the
    waits onto single-wait SP nops emitted between the drain and barrier."""
    if getattr(tile.TileContext, "_tail_drain_patched", False):
        return

    def _drain_and_barrier(self, tick_clock, wait_clock):
        from concourse.tile import ScopedClock

        drain_inst = self.nc.sync.drain()
        wait_clock.add_sem_waits(
            drain_inst.ins, ScopedClock({None: tick_clock.global_clock})
        )
        waits = list(drain_inst.ins.sync_info.on_wait or [])
        if len(waits) > 1:
            drain_inst.ins.sync_info = mybir.SyncInfo(
                on_wait=[waits[0]], on_update=[]
            )
            for w in waits[1:]:
                nop = self.nc.sync.nop()
                nop.ins.sync_info = mybir.SyncInfo(on_wait=[w], on_update=[])
        self.nc.all_engine_barrier()
        assert self.sems is not None
        popped = self.nc._tile_sem_poison_stack.pop()
        assert popped is self._sem_poison
        self.nc.clear_and_free_semaphores(list(self.sems.allocated().values()))
        self.nc.all_engine_barrier()

    tile.TileContext._drain_and_barrier = _drain_and_barrier
    tile.TileContext._tail_drain_patched = True


def _split_multi_waits(bir_json: bytes) -> bytes:
    """This container's walrus supports only ONE sync-wait slot per
    instruction ("Too many sync wait commands").  Hoist extra waits onto
    single-wait NoOps inserted just before the instruction (same engine, so
    per-engine program order and blocking semantics are preserved)."""
    d = json.loads(bir_json)
    n = 0
    for f in d.get("functions", []):
        for bb in f.get("blocks", []):
            insts = bb.get("instructions", [])
            out = []
            changed = False
            for inst in insts:
                si = inst.get("sync_info")
                waits = (si.get("on_wait") or []) if si else []
                if len(waits) > 1:
                    changed = True
                    for w in waits[:-1]:
                        n += 1
                        out.append({
                            "debug": inst.get("debug"),
                            "engine": inst["engine"],
                            "ins": [],
                            "name": f"WSPLIT-{n}",
                            "opcode": "NoOp",
                            "outs": [],
                            "sync_info": {"on_update": [], "on_wait": [w]},
                        })
                    si["on_wait"] = [waits[-1]]
                out.append(inst)
            if changed:
                bb["instructions"] = out
    if n == 0:
        return bir_json
    return json.dumps(d).encode()


def _patched_compile_bir_kernel(bir_json, tmpdir, neff_name="file.neff"):
    if isinstance(bir_json, str):
        bir_json = bir_json.encode()
    return _orig_compile_bir_kernel(_split_multi_waits(bir_json), tmpdir, neff_name)


bass2jax.compile_bir_kernel = _patched_compile_bir_kernel


def build_nc(nb=NB):
    _patch_tile_tail_drain()
    nc = bass.Bass(trn_type="TRN2")

    xt = nc.dram_tensor("xt", [E, nb * T], BF16, kind="ExternalInput")
    # a_pad: [256, 256] with A in [0:192, 0:192], zeros elsewhere
    a = nc.dram_tensor("a", [256, 256], BF16, kind="ExternalInput")
    # wvt_pad: [256, H] with Wv.T in [0:192, :], zeros below
    wvt = nc.dram_tensor("wvt", [256, H], BF16, kind="ExternalInput")
    o = nc.dram_tensor("o", [T, nb, H], BF16, kind="ExternalOutput")

    nq = nb // QUAD
    nwarm = int(os.environ.get("KWARM", "0"))
    is_ge = mybir.AluOpType.is_ge

    with tile.TileContext(nc) as tc, ExitStack() as ctx:
        singles = ctx.enter_context(tc.tile_pool(name="singles", bufs=1))
        px = ctx.enter_context(tc.tile_pool(name="px", bufs=3))
        pgsb = ctx.enter_context(tc.tile_pool(name="pgsb", bufs=2))
        ppm = ctx.enter_context(tc.tile_pool(name="ppm", bufs=2))
        pvsb = ctx.enter_context(tc.tile_pool(name="pvsb", bufs=4))
        posb = ctx.enter_context(tc.tile_pool(name="posb", bufs=2))
        psr = ctx.enter_context(tc.tile_pool(name="psr", bufs=2))

        pglo = ctx.enter_context(tc.tile_pool(name="pglo", bufs=1, space="PSUM"))
        pghi = ctx.enter_context(tc.tile_pool(name="pghi", bufs=1, space="PSUM"))
        pw = ctx.enter_context(tc.tile_pool(name="pw", bufs=2, space="PSUM"))
        pv = ctx.enter_context(tc.tile_pool(name="pv", bufs=2, space="PSUM"))
        po = ctx.enter_context(tc.tile_pool(name="po", bufs=1, space="PSUM"))

        # Constants (all K=128-padded).
        a_lo = singles.tile([128, 256], BF16, tag="a_lo")
        a_hi = singles.tile([128, 256], BF16, tag="a_hi")
        nc.sync.dma_start(out=a_lo, in_=a[0:128, :])
        nc.sync.dma_start(out=a_hi, in_=a[128:256, :])
        wvt_lo = singles.tile([128, H], BF16, tag="wvt_lo")
        wvt_hi = singles.tile([128, H], BF16, tag="wvt_hi")
        nc.sync.dma_start(out=wvt_lo, in_=wvt[0:128, :])
        nc.sync.dma_start(out=wvt_hi, in_=wvt[128:256, :])

        # xhi rotation buffers: partitions 0-63 get DMA'd per group,
        # partitions 64-127 are zeroed once (K=128 padding).
        xhi_bufs = []
        for k in range(3):
            xb = singles.tile([128, G * T], BF16, tag=f"xhi{k}",
                              name=f"xhi{k}")
            nc.gpsimd.memset(xb[64:128, :], 0.0)
            xhi_bufs.append(xb)

        x_tiles = {}   # group -> (xlo, xhi)
        gsb_t = {}     # Q -> (glo, ghi, qs) then (gsb_lo, gsb_hi, qs)
        wei_t = {}     # Q -> wei PSUM tile
        pm_t = {}      # Q -> pm SBUF tile
        vsb_t = {}     # Q -> [v_sb pair0, v_sb pair1]
        osb_t = {}     # group -> o_sb

        def emit_gt(Q):
            g = Q * QUAD // G
            if (Q * QUAD) % G == 0:
                gcol = g * G * T
                xlo = px.tile([128, G * T], BF16, tag="xlo", name="xlo")
                xhi = xhi_bufs[g % 3]
                nc.sync.dma_start(out=xlo, in_=xt[0:128, gcol : gcol + G * T])
                nc.sync.dma_start(out=xhi[0:64, :],
                                  in_=xt[128:192, gcol : gcol + G * T])
                x_tiles[g] = (xlo, xhi)
            xlo, xhi = x_tiles[g]
            qs = (Q * QUAD * T) % (G * T)
            qcols = slice(qs, qs + QUAD * T)
            glo = pglo.tile([128, QUAD * T], F32, tag="glo", name="glo")
            ghi = pghi.tile([128, QUAD * T], F32, tag="ghi", name="ghi")
            if Q == 0 and nwarm:
                for _ in range(nwarm):
                    nc.tensor.matmul(glo, a_lo[:, 0:128], xlo[:, qcols],
                                     start=True, stop=True)
            nc.tensor.matmul(glo, a_lo[:, 0:128], xlo[:, qcols],
                             start=True, stop=False)
            nc.tensor.matmul(glo, a_hi[:, 0:128], xhi[:, qcols],
                             start=False, stop=True)
            nc.tensor.matmul(ghi, a_lo[:, 128:256], xlo[:, qcols],
                             start=True, stop=False)
            nc.tensor.matmul(ghi, a_hi[:, 128:256], xhi[:, qcols],
                             start=False, stop=True)
            gsb_t[Q] = (glo, ghi, qs)

        def emit_gsb_copies(Q):
            glo, ghi, qs = gsb_t[Q]
            gsb_lo = pgsb.tile([128, QUAD * T], BF16, tag="gsb_lo",
                               name="gsb_lo")
            gsb_hi = pgsb.tile([128, QUAD * T], BF16, tag="gsb_hi",
                               name="gsb_hi")
            nc.scalar.copy(out=gsb_lo, in_=glo)
            nc.vector.tensor_copy(out=gsb_hi, in_=ghi)
            gsb_t[Q] = (gsb_lo, gsb_hi, qs)

        def emit_expmask(Q):
            wei = wei_t.pop(Q)
            pm = ppm.tile([128, QUAD, T], BF16, tag="pm", name="pm")
            nc.scalar.activation(out=pm, in_=wei,
                                 func=mybir.ActivationFunctionType.Exp)
            # causal keep where q - k >= 0 (k = partition, q = inner free dim)
            nc.gpsimd.affine_select(
                out=pm, in_=pm, compare_op=is_ge, fill=0.0, base=0,
                pattern=[[0, QUAD], [1, T]], channel_multiplier=-1,
            )
            pm_t[Q] = pm

        def emit_vwei(Q):
            g = Q * QUAD // G
            xlo, xhi = x_tiles[g]
            gsb_lo, gsb_hi, qs = gsb_t.pop(Q)
            wei = pw.tile([128, QUAD, T], F32, tag="wei", name="wei")
            vsb_t[Q] = []
            for pr in range(2):
                v_ps = pv.tile([128, 2, HP], F32, tag="v_ps", name="v_ps")
                for jj in range(2):
                    j = pr * 2 + jj
                    bs = qs + j * T
                    jc = slice(j * T, (j + 1) * T)
                    nc.tensor.matmul(v_ps[:, jj, 0:H], xlo[:, bs : bs + T],
                                     wvt_lo, start=True, stop=False)
                    nc.tensor.matmul(v_ps[:, jj, 0:H], xhi[:, bs : bs + T],
                                     wvt_hi, start=False, stop=True)
                    nc.tensor.matmul(wei[:, j, :], xlo[:, bs : bs + T],
                                     gsb_lo[:, jc], start=True, stop=False)
                    nc.tensor.matmul(wei[:, j, :], xhi[:, bs : bs + T],
                                     gsb_hi[:, jc], start=False, stop=True)
                v_sb = pvsb.tile([128, 2, HP], BF16, tag="v_sb", name="v_sb")
                if pr == 0:
                    nc.scalar.copy(out=v_sb[:, :, 0:H], in_=v_ps[:, :, 0:H])
                else:
                    nc.vector.tensor_copy(out=v_sb[:, :, 0:H],
                                          in_=v_ps[:, :, 0:H])
                nc.gpsimd.memset(v_sb[:, :, H : H + 1], 1.0)
                vsb_t[Q].append(v_sb)
            wei_t[Q] = wei

        def emit_o(Q):
            pm = pm_t.pop(Q)
            o_ps = po.tile([128, 2, 2, 256], F32, tag="o_ps", name="o_ps")
            for pr in range(2):
                v_sb = vsb_t[Q][pr]
                for jj in range(2):
                    j = pr * 2 + jj
                    nc.tensor.matmul(o_ps[:, pr, jj, 0 : H + 1], pm[:, j, :],
                                     v_sb[:, jj, 0 : H + 1],
                                     start=True, stop=True)
            del vsb_t[Q]
            gb = Q * QUAD // G
            ob0 = (Q * QUAD) % G
            if ob0 == 0:
                osb_t[gb] = posb.tile([128, G, H], BF16, tag="o_sb",
                                      name="o_sb")
            o_sb = osb_t[gb]
            r = psr.tile([128, 2, 2], F32, tag="r", name="r")
            nc.vector.reciprocal(out=r, in_=o_ps[:, :, :, H])
            rb = r[:, :, :].unsqueeze(3).broadcast_to([128, 2, 2, H])
            out_ap = o_sb[:, ob0 : ob0 + QUAD, :].rearrange(
                "p (a b) h -> p a b h", a=2
            )
            nc.vector.tensor_mul(out_ap, o_ps[:, :, :, 0:H], rb)
            if ob0 + QUAD == G:
                nc.sync.dma_start(out=o[:, gb * G : (gb + 1) * G, :],
                                  in_=o_sb)
                del osb_t[gb]

        emit_gt(0)
        emit_gsb_copies(0)
        for i in range(nq + 1):
            if i + 1 < nq:
                emit_gt(i + 1)
            if i >= 1:
                emit_expmask(i - 1)
            if i + 1 < nq:
                emit_gsb_copies(i + 1)
            if i < nq:
                emit_vwei(i)
            if i >= 1:
                emit_o(i - 1)
    return nc


_cached = {}


def _get_nc(nb):
    if nb not in _cached:
        _cached[nb] = build_nc(nb)
    return _cached[nb]


def prep_inputs(x, Wq, Wk, Wv, nb=NB, ncores=NCORES):
    """Host-side sharding + layout/dtype prep + weight folding + padding."""
    x = np.asarray(x, dtype=np.float32)
    A = (np.asarray(Wq, np.float32).T @ np.asarray(Wk, np.float32)) * SCALE
    a_pad = np.zeros((256, 256), np.float32)
    a_pad[:E, :E] = A
    a_bf = a_pad.astype(NPBF16)
    wvt_pad = np.zeros((256, H), np.float32)
    wvt_pad[:E, :] = np.asarray(Wv, np.float32).T
    wvt_bf = wvt_pad.astype(NPBF16)
    in_maps = []
    for c in range(ncores):
        shard = x[c * nb : (c + 1) * nb]                      # [nb, T, E]
        xtc = np.ascontiguousarray(shard.transpose(2, 0, 1)).reshape(
            E, nb * T
        )
        in_maps.append({"xt": xtc.astype(NPBF16), "a": a_bf, "wvt": wvt_bf})
    return in_maps


def kernel(x, Wq, Wk, Wv, _trace=False, _tmpdir=None):
    nc = _get_nc(NB)
    in_maps = prep_inputs(x, Wq, Wk, Wv)
    res = run_bass_kernel_spmd(
        nc, in_maps, core_ids=list(range(NCORES)), trace=_trace, tmpdir=_tmpdir
    )
    nb = NB
    out = np.empty((B, T, H), np.float32)
    for c in range(NCORES):
        out[c * nb : (c + 1) * nb] = (
            res.results[c]["o"].transpose(1, 0, 2).astype(np.float32)
        )
    if _trace:
        kernel.last_result = res
    return out
